# revision 1
# baseline (speedup 1.0000x reference)
"""Trainium2 Bass kernel for nn_Attention_36361193128703 (self-contained).

Entry point: kernel(**inputs) -> np.ndarray
  inputs: x (2,2048,1024) f32, w_in (3072,1024) f32,
          kernel_offsets/amplitudes/sharpness (16,16) f32
  returns: (2, 2048, 1024) f32 attention output (matches reference).

Distribution: 8 NeuronCores = data-parallel over batch (2) x tensor-parallel
over heads (4 head-groups of 4). Each core runs an identical single-core Bass
program on its shard; outputs are concatenated on the host. No collectives.
"""
from contextlib import ExitStack

import numpy as np

import concourse.bass as bass
import concourse.mybir as mybir
import concourse.tile as tile
from concourse import bacc
from concourse.bass import AP
from concourse.masks import make_identity

F32 = mybir.dt.float32
BF16 = mybir.dt.bfloat16
I32 = mybir.dt.int32

L = 2048
DM = 1024
HL = 4            # local heads
HD = 64
M = 4608          # padded score length (>= 2L-1 = 4095), 9 x 512 chunks
GW = 4096         # g_rep free width (window reach 3968; 8 x 512 flip chunks)
IC = 1024         # i-chunk (query half) width for the attention phase
NIC = L // IC     # 2
JT = 128          # j-tile (key) height
NJT = L // JT     # 16
NDC = DM // 128   # 8 d-chunks


def build_kernel() -> bacc.Bacc:
    nc = bacc.Bacc("TRN2", target_bir_lowering=False, debug=False, num_devices=8)

    xT_d = nc.dram_tensor("xT", [DM, L], BF16, kind="ExternalInput")
    wkqv_d = nc.dram_tensor("wkqv", [DM, 768], BF16, kind="ExternalInput")
    tisa_d = nc.dram_tensor("tisa", [64, 6], F32, kind="ExternalInput")
    out_d = nc.dram_tensor("out", [L, 256], F32, kind="ExternalOutput")

    # round-robin DMA issuing engines so loads spread over more HW queues
    dma_engines = [nc.sync, nc.scalar, nc.gpsimd]

    def dma(i, out, in_):
        dma_engines[i % len(dma_engines)].dma_start(out, in_)

    with tile.TileContext(nc) as tc, ExitStack() as ctx:
        const_pool = ctx.enter_context(tc.tile_pool(name="const", bufs=1))

        # one shared startup/aux PSUM pool: scores + proj + flip share a single
        # 1-bank slot (tag aux); epilogue transposes get their own 1-bank slot.
        aux_psum = ctx.enter_context(tc.tile_pool(name="auxps", bufs=2, space="PSUM"))
        s_psum = ctx.enter_context(tc.tile_pool(name="sps", bufs=2, space="PSUM"))
        o_psum = ctx.enter_context(tc.tile_pool(name="ops", bufs=1, space="PSUM"))

        # ---------------- Phase 0: TISA scores -> g_pad (DRAM, bf16) -------------
        gdram_pool = ctx.enter_context(tc.tile_pool(name="gdram", bufs=1, space="DRAM"))
        g_pad = gdram_pool.tile([HL * M], BF16)

        with tc.tile_pool(name="tisa_tmp", bufs=1) as tp:
            tisa_sb = tp.tile([64, 6], F32)
            nc.sync.dma_start(tisa_sb[:, :], tisa_d[:, :])
            abs_sh = tp.tile([64, 1], F32)
            nc.scalar.activation(abs_sh[:, :], tisa_sb[:, 1:2],
                                 mybir.ActivationFunctionType.Abs)
            # single in-place scratch: rel -> (rel-off)^2*|sharp| -> exp(-.)
            # chunked so iota/DVE/ACT pipeline instead of serializing
            ev = tp.tile([64, M], F32, tag="scr")
            evb = tp.tile([64, M], BF16, tag="scrb")
            ampb = tp.tile([64, 4], BF16)
            nc.vector.tensor_copy(ampb[:, :], tisa_sb[:, 2:6])
            CH = M // 3
            for cc in range(3):
                sl = slice(cc * CH, (cc + 1) * CH)
                nc.gpsimd.iota(ev[:, sl], pattern=[[1, CH]],
                               base=-(L - 1) + cc * CH,
                               channel_multiplier=0,
                               allow_small_or_imprecise_dtypes=True)
                nc.vector.tensor_scalar(ev[:, sl], ev[:, sl], tisa_sb[:, 0:1],
                                        None, op0=mybir.AluOpType.subtract)
                nc.vector.tensor_mul(ev[:, sl], ev[:, sl], ev[:, sl])
                nc.vector.tensor_scalar(ev[:, sl], ev[:, sl], abs_sh[:, 0:1],
                                        None, op0=mybir.AluOpType.mult)
                nc.scalar.activation(evb[:, sl], ev[:, sl],
                                     mybir.ActivationFunctionType.Exp,
                                     scale=-1.0)
            if True:
                for mc in range(M // 512):
                    ps = o_psum.tile([65, IC], F32, tag="O", name="ps")
                    nc.tensor.matmul(ps[0:HL, 0:512], ampb[:, :],
                                     evb[:, mc * 512:(mc + 1) * 512],
                                     start=True, stop=True)
                    gch = tp.tile([HL, 512], BF16, tag="gch")
                    nc.scalar.activation(gch[:, :], ps[0:HL, 0:512],
                                         mybir.ActivationFunctionType.Exp)
                    gbase = g_pad[:]
                    dst = AP(gbase.tensor, gbase.offset + mc * 512,
                             [[M, HL], [1, 512]])
                    nc.sync.dma_start(dst, gch[:, :])

        # consts built after the scores chain so gpsimd's iota starts first
        ident = const_pool.tile([128, 128], F32)
        make_identity(nc, ident[:, :])
        # anti-identity: anti[c, p] = 1 iff c + p == 127 (partition-flip matmul)
        anti = const_pool.tile([128, 128], BF16)
        nc.gpsimd.memset(anti[:, :], 0.0)
        nc.gpsimd.affine_select(
            out=anti[:, :], in_=anti[:, :],
            compare_op=mybir.AluOpType.not_equal, fill=1.0,
            base=-127, channel_multiplier=1, pattern=[[1, 128]])

        # ---------------- Phase 1: load inputs; projections (scoped pools) -------
        kq_pool = ctx.enter_context(tc.tile_pool(name="kq", bufs=1))
        v_pool = ctx.enter_context(tc.tile_pool(name="V", bufs=1))
        kq_sb = [[kq_pool.tile([128, 512], BF16, name=f"kq{i}_{t}",
                          tag=f"kq{i}_{t}") for t in range(4)]
                 for i in range(4)]
        v_sb = []
        xpool = ctx.enter_context(tc.tile_pool(name="xT", bufs=1))
        wpool = ctx.enter_context(tc.tile_pool(name="w", bufs=1))
        if True:
            xT_sb = []
            wkq_sb = []
            wv_sb = []
            for dc in range(NDC):
                xt = xpool.tile([128, L], BF16, name=f"xt{dc}", tag=f"xt{dc}")
                dma(2 * dc, xt[:, 0:L // 2],
                    xT_d[dc * 128:(dc + 1) * 128, 0:L // 2])
                dma(2 * dc + 1, xt[:, L // 2:L],
                    xT_d[dc * 128:(dc + 1) * 128, L // 2:L])
                xT_sb.append(xt)
            for dc in range(NDC):
                wt = wpool.tile([128, 768], BF16, name=f"wkqv{dc}",
                                tag=f"wkqv{dc}")
                dma(dc + 1, wt[:, :], wkqv_d[dc * 128:(dc + 1) * 128, :])
                wkq_sb.append(wt[:, 0:512])
                wv_sb.append(wt[:, 512:768])

            def emit_kq(ec, copy_eng="act"):
                for tcn in range(4):
                    ps = aux_psum.tile([128, 512], F32, tag="aux", name="ps")
                    for k in range(NDC):
                        dc = (tcn * 2 + k) % NDC
                        nc.tensor.matmul(ps[:, :],
                                         wkq_sb[dc][:, ec * 128:(ec + 1) * 128],
                                         xT_sb[dc][:, tcn * 512:(tcn + 1) * 512],
                                         start=(k == 0), stop=(k == NDC - 1))
                    if copy_eng == "act":
                        nc.scalar.copy(kq_sb[ec][tcn][:, :], ps[:, :])
                    else:
                        nc.vector.tensor_copy(kq_sb[ec][tcn][:, :], ps[:, :])

            def emit_vproj():
                for tt in range(NJT):
                    ps = aux_psum.tile([128, 512], F32, tag="aux", name="ps")
                    for dc in range(NDC):
                        nc.tensor.matmul(ps[:, 0:256],
                                         xT_sb[dc][:, tt * 128:(tt + 1) * 128],
                                         wv_sb[dc][:, :],
                                         start=(dc == 0), stop=(dc == NDC - 1))
                    vt = v_pool.tile([128, 4 * 65], BF16, name=f"v{tt}", tag=f"v{tt}")
                    for hi in range(HL):
                        nc.vector.tensor_copy(vt[:, hi * 65:hi * 65 + 64],
                                              ps[:, hi * 64:(hi + 1) * 64])
                        nc.vector.memset(vt[:, hi * 65 + 64:hi * 65 + 65], 1.0)
                    v_sb.append(vt)

        # ---------------- Phase 2: g_rep build (PE partition-flip) ----------------
        # srep[c, t] = g[h, t + c] loads with positive strides; the anti-identity
        # matmul flips partitions: grep[p, t] = srep[127-p, t] = g[h, t + 127 - p].
        grep_pool = ctx.enter_context(tc.tile_pool(name="grep", bufs=1))
        srp = ctx.enter_context(tc.tile_pool(name="srep", bufs=2))
        grep_sb = [None] * HL
        gbase = g_pad[:]

        def emit_flip(hi):
            sr = srp.tile([128, GW], BF16, tag="sr", name=f"sr{hi}")
            src = AP(gbase.tensor, gbase.offset + hi * M, [[1, 128], [1, GW]])
            dma(hi, sr[:, :], src)
            gr = grep_pool.tile([128, GW], BF16, name=f"grep{hi}",
                                tag=f"grep{hi}")
            for fc in range(GW // 512):
                fps = aux_psum.tile([128, 512], F32, tag="aux", name="fps")
                nc.tensor.matmul(fps[:, :], anti[:, :],
                                 sr[:, fc * 512:(fc + 1) * 512],
                                 start=True, stop=True)
                nc.vector.tensor_copy(gr[:, fc * 512:(fc + 1) * 512],
                                      fps[:, :])
            grep_sb[hi] = gr

        # ---------------- Phase 3: attention (i-half software pipeline) ----------
        p_pool = ctx.enter_context(tc.tile_pool(name="p", bufs=1))
        e_pool = ctx.enter_context(tc.tile_pool(name="es", bufs=4))
        o_pool = ctx.enter_context(tc.tile_pool(name="o", bufs=2))
        r_pool = ctx.enter_context(tc.tile_pool(name="r", bufs=2))
        out_pool = ctx.enter_context(tc.tile_pool(name="out", bufs=3))

        def emit_P(hi, i0):
            """S matmuls + exp + g-mult for one (head, i-half) -> P tiles in SBUF."""
            kqt = kq_sb[hi // 2]
            qqt = kq_sb[2 + hi // 2]
            pb = (hi % 2) * 64
            tiles = []
            for jt in range(NJT):
                j0 = jt * JT
                t0 = (L - 1 - 127) - j0 + i0
                ps_s = s_psum.tile([128, IC], F32, tag="S", name=f"ps_s{jt}")
                for f2 in range(IC // 512):
                    iq = i0 + f2 * 512
                    nc.tensor.matmul(
                        ps_s[:, f2 * 512:(f2 + 1) * 512],
                        kqt[j0 // 512][pb:pb + 64, j0 % 512:j0 % 512 + JT],
                        qqt[iq // 512][pb:pb + 64, :],
                        start=True, stop=True)
                es = e_pool.tile([128, IC], BF16, tag="es", name=f"es{jt}")
                nc.scalar.activation(es[:, :], ps_s[:, :],
                                     mybir.ActivationFunctionType.Exp)
                pt = p_pool.tile([128, IC], BF16, tag=f"p{jt}", name=f"p{jt}")
                nc.vector.tensor_mul(pt[:, :], es[:, :],
                                     grep_sb[hi][:, t0:t0 + IC])
                tiles.append(pt)
            return tiles

        def emit_AV(hi, i0, p_tiles, final=False):
            """AV accumulation + normalize + store for one (head, i-half)."""
            ps_o = o_psum.tile([65, IC], F32, tag="O", name="ps_o")
            for jt in range(NJT):
                for f2 in range(IC // 512):
                    nc.tensor.matmul(
                        ps_o[:, f2 * 512:(f2 + 1) * 512],
                        v_sb[jt][:, hi * 65:hi * 65 + 65],
                        p_tiles[jt][:, f2 * 512:(f2 + 1) * 512],
                        start=(jt == 0), stop=(jt == NJT - 1))
            o_sb = o_pool.tile([65, IC], F32, tag="O", name="o_sb")
            nc.vector.tensor_copy(o_sb[:, :], ps_o[:, :])
            for tq in range(IC // 128):
                ps_t = aux_psum.tile([128, 512], F32, tag="aux", name="ps_t")
                nc.tensor.transpose(ps_t[:, 0:65],
                                    o_sb[:, tq * 128:(tq + 1) * 128],
                                    ident[0:65, 0:65])
                rc = r_pool.tile([128, 1], F32, tag="rc", name="rc")
                nc.vector.reciprocal(rc[:, :], ps_t[:, 64:65])
                ot = out_pool.tile([128, HD], F32, tag="ot", name="ot")
                nc.vector.tensor_scalar(ot[:, :], ps_t[:, 0:64], rc[:, 0:1],
                                        None, op0=mybir.AluOpType.mult)
                nc.sync.dma_start(
                    out_d[i0 + tq * 128:i0 + (tq + 1) * 128,
                          hi * HD:(hi + 1) * HD],
                    ot[:, :])

        # interleaved emission: enough proj/flip for heads 0-1, first P phase,
        # then the rest of proj/V/flips, then the remaining pipeline.
        emit_kq(0)
        emit_kq(2)
        emit_flip(0)
        emit_flip(1)
        cur = emit_P(0, 0)
        prev = (0, 0, cur)
        emit_vproj()
        emit_kq(1, copy_eng="dve")
        emit_kq(3, copy_eng="dve")
        emit_flip(2)
        emit_flip(3)
        for hi in range(HL):
            for half in range(NIC):
                if hi == 0 and half == 0:
                    continue
                i0 = half * IC
                cur = emit_P(hi, i0)
                emit_AV(prev[0], prev[1], prev[2])
                prev = (hi, i0, cur)
        emit_AV(prev[0], prev[1], prev[2], final=True)

    nc.compile()
    return nc


def shard_inputs(inputs: dict) -> list[dict]:
    """Full inputs -> 8 per-core input maps (bf16 prep for matmul operands)."""
    import ml_dtypes

    x, w_in = inputs["x"], inputs["w_in"]
    off = inputs["kernel_offsets"]
    amp = inputs["kernel_amplitudes"]
    sh = inputs["kernel_sharpness"]
    D = DM
    in_maps = []
    for c in range(8):
        b, hg = c // 4, c % 4
        heads = list(range(4 * hg, 4 * hg + 4))
        xT = np.ascontiguousarray(x[b].T).astype(ml_dtypes.bfloat16)
        rows_k = np.concatenate([w_in[h * HD:(h + 1) * HD] for h in heads])
        rows_q = np.concatenate(
            [w_in[2 * D + h * HD:2 * D + (h + 1) * HD] for h in heads]
        ) * np.float32(1.0 / np.sqrt(HD))
        rows_v = np.concatenate([w_in[D + h * HD:D + (h + 1) * HD] for h in heads])
        wkqv = np.ascontiguousarray(
            np.concatenate([np.concatenate([rows_k, rows_q]).T, rows_v.T],
                           axis=1)).astype(ml_dtypes.bfloat16)
        tisa = np.zeros((64, 6), np.float32)
        tisa[:, 0] = off[heads].reshape(-1)
        tisa[:, 1] = sh[heads].reshape(-1)
        for hi in range(4):
            tisa[hi * 16:(hi + 1) * 16, 2 + hi] = amp[heads[hi]]
        in_maps.append({"xT": xT, "wkqv": wkqv, "tisa": tisa})
    return in_maps


def unshard_output(results: list[dict]) -> np.ndarray:
    out = np.zeros((2, L, DM), np.float32)
    for c in range(8):
        b, hg = c // 4, c % 4
        out[b, :, hg * 256:(hg + 1) * 256] = results[c]["out"]
    return out


_NC_CACHE = None


def kernel(**inputs) -> np.ndarray:
    global _NC_CACHE
    from concourse.bass_utils import run_bass_kernel_spmd

    if _NC_CACHE is None:
        _NC_CACHE = build_kernel()
    in_maps = shard_inputs({k: np.asarray(v) for k, v in inputs.items()})
    res = run_bass_kernel_spmd(_NC_CACHE, in_maps, core_ids=list(range(8)))
    return unshard_output(res.results)



# revision 6
# speedup vs baseline: 1.1247x; 1.1247x over previous
"""Trainium2 Bass kernel for nn_Attention_36361193128703 (self-contained).

Entry point: kernel(**inputs) -> np.ndarray
  inputs: x (2,2048,1024) f32, w_in (3072,1024) f32,
          kernel_offsets/amplitudes/sharpness (16,16) f32
  returns: (2, 2048, 1024) f32 attention output (matches reference).

Distribution: 8 NeuronCores = data-parallel over batch (2) x tensor-parallel
over heads (4 head-groups of 4). Each core runs an identical single-core Bass
program on its shard; outputs are concatenated on the host. No collectives.

Core pipeline (per core: 4 heads = 2 head-pairs, L=2048):
  - TISA scores -> g = exp(bias) staged in DRAM, then loaded as 128 shifted
    copies per head via a negative-partition-stride DMA (Toeplitz expansion).
  - S = k^T q for a head-pair: two K=64 matmuls packed into the PE array
    concurrently (row groups 0-63 / 64-127) writing one [128, 2, 512] PSUM
    pair tile.
  - P = exp(S/8) * g: one ACT exp (FD=1024) + one DVE multiply (2x mode).
  - O = V^T P accumulated in PSUM with a ones-column for the denominator.
  - Epilogue: PE transpose to [query, 64] layout, DVE reciprocal+scale, DMA.
"""
from contextlib import ExitStack

import numpy as np

import concourse.bass as bass
import concourse.mybir as mybir
import concourse.tile as tile
from concourse import bacc
from concourse.bass import AP
from concourse.masks import make_identity

F32 = mybir.dt.float32
BF16 = mybir.dt.bfloat16

L = 2048
DM = 1024
HL = 4            # local heads
HD = 64
M = 4608          # padded score length (>= 2L-1 = 4095), 9 x 512 chunks
GW = 4096         # g window width per head (max needed index 4094)
IC = 512          # i-chunk (query) width per unit
NCH = L // IC     # 4 chunks
JT = 128          # j-tile (key) height
NJT = L // JT     # 16
NDC = DM // 128   # 8 d-chunks


def build_kernel() -> bacc.Bacc:
    nc = bacc.Bacc("TRN2", target_bir_lowering=False, debug=False, num_devices=8)

    xT_d = nc.dram_tensor("xT", [DM, L], BF16, kind="ExternalInput")
    wkqv_d = nc.dram_tensor("wkqv", [DM, 768], BF16, kind="ExternalInput")
    tisa_d = nc.dram_tensor("tisa", [64, 6], F32, kind="ExternalInput")
    out_d = nc.dram_tensor("out", [L, 256], F32, kind="ExternalOutput")

    dma_engines = [nc.sync, nc.gpsimd]

    def dma(i, out, in_):
        dma_engines[i % len(dma_engines)].dma_start(out, in_)

    with tile.TileContext(nc) as tc, ExitStack() as ctx:
        const_pool = ctx.enter_context(tc.tile_pool(name="const", bufs=1))

        aux_psum = ctx.enter_context(tc.tile_pool(name="auxps", bufs=2, space="PSUM"))
        s_psum = ctx.enter_context(tc.tile_pool(name="sps", bufs=2, space="PSUM"))
        o_psum = ctx.enter_context(tc.tile_pool(name="ops", bufs=1, space="PSUM"))

        # ---------------- Phase 0: TISA scores -> g_pad (DRAM, bf16) -------------
        gdram_pool = ctx.enter_context(tc.tile_pool(name="gdram", bufs=1, space="DRAM"))
        g_pad = gdram_pool.tile([HL * M], BF16)

        with tc.tile_pool(name="tisa_tmp", bufs=1) as tp:
            tisa_sb = tp.tile([64, 6], F32)
            nc.sync.dma_start(tisa_sb[:, :], tisa_d[:, :])
            abs_sh = tp.tile([64, 1], F32)
            nc.scalar.activation(abs_sh[:, :], tisa_sb[:, 1:2],
                                 mybir.ActivationFunctionType.Abs)
            # single in-place scratch: rel -> (rel-off)^2*|sharp| -> exp(-.)
            # chunked so iota/DVE/ACT pipeline instead of serializing
            ev = tp.tile([64, M], F32, tag="scr")
            evb = tp.tile([64, M], BF16, tag="scrb")
            ampb = tp.tile([64, 4], BF16)
            nc.vector.tensor_copy(ampb[:, :], tisa_sb[:, 2:6])
            # reversed relative positions: ev[:, m] = (L-1) - m, so g_pad holds
            # rev[m] = score[2L-2 - m]; the Toeplitz flip then needs only
            # positive DMA strides (+ a reversed free-dim read in the multiply).
            CH = M // 3
            for cc in range(3):
                sl = slice(cc * CH, (cc + 1) * CH)
                nc.gpsimd.iota(ev[:, sl], pattern=[[-1, CH]],
                               base=(L - 1) - cc * CH,
                               channel_multiplier=0,
                               allow_small_or_imprecise_dtypes=True)
                nc.vector.tensor_scalar(ev[:, sl], ev[:, sl], tisa_sb[:, 0:1],
                                        None, op0=mybir.AluOpType.subtract)
                nc.vector.tensor_mul(ev[:, sl], ev[:, sl], ev[:, sl])
                nc.vector.tensor_scalar(ev[:, sl], ev[:, sl], abs_sh[:, 0:1],
                                        None, op0=mybir.AluOpType.mult)
                nc.scalar.activation(evb[:, sl], ev[:, sl],
                                     mybir.ActivationFunctionType.Exp,
                                     scale=-1.0)
            for mc in range(M // 512):
                ps = o_psum.tile([65, IC], F32, tag="O0", name="ps")
                nc.tensor.matmul(ps[0:HL, 0:512], ampb[:, :],
                                 evb[:, mc * 512:(mc + 1) * 512],
                                 start=True, stop=True)
                gch = tp.tile([HL, 512], BF16, tag="gch")
                nc.scalar.activation(gch[:, :], ps[0:HL, 0:512],
                                     mybir.ActivationFunctionType.Exp)
                gbase = g_pad[:]
                dst = AP(gbase.tensor, gbase.offset + mc * 512,
                         [[M, HL], [1, 512]])
                nc.sync.dma_start(dst, gch[:, :])

        ident = const_pool.tile([128, 128], F32)
        make_identity(nc, ident[:, :])

        # ---------------- Phase 1: load inputs; projections ----------------------
        kq_pool = ctx.enter_context(tc.tile_pool(name="kq", bufs=1))
        v_pool = ctx.enter_context(tc.tile_pool(name="V", bufs=1))
        kq_sb = [[kq_pool.tile([128, 512], BF16, name=f"kq{i}_{t}",
                          tag=f"kq{i}_{t}") for t in range(4)]
                 for i in range(4)]
        v_sb = []
        xpool = ctx.enter_context(tc.tile_pool(name="xT", bufs=1))
        wpool = ctx.enter_context(tc.tile_pool(name="w", bufs=1))
        xT_sb = []
        wkq_sb = []
        wv_sb = []
        for dc in range(NDC):
            xt = xpool.tile([128, L], BF16, name=f"xt{dc}", tag=f"xt{dc}")
            dma(2 * dc, xt[:, 0:L // 2],
                xT_d[dc * 128:(dc + 1) * 128, 0:L // 2])
            dma(2 * dc + 1, xt[:, L // 2:L],
                xT_d[dc * 128:(dc + 1) * 128, L // 2:L])
            xT_sb.append(xt)
        for dc in range(NDC):
            wt = wpool.tile([128, 768], BF16, name=f"wkqv{dc}",
                            tag=f"wkqv{dc}")
            dma(dc + 1, wt[:, :], wkqv_d[dc * 128:(dc + 1) * 128, :])
            wkq_sb.append(wt[:, 0:512])
            wv_sb.append(wt[:, 512:768])

        def emit_kq_tcn(ec, tcn, copy_eng):
            ps = aux_psum.tile([128, 512], F32, tag="aux", name="ps")
            for k in range(NDC):
                dc = (tcn * 2 + k) % NDC
                nc.tensor.matmul(ps[:, :],
                                 wkq_sb[dc][:, ec * 128:(ec + 1) * 128],
                                 xT_sb[dc][:, tcn * 512:(tcn + 1) * 512],
                                 start=(k == 0), stop=(k == NDC - 1))
            if copy_eng == "act":
                nc.scalar.copy(kq_sb[ec][tcn][:, :], ps[:, :])
            else:
                nc.vector.tensor_copy(kq_sb[ec][tcn][:, :], ps[:, :])

        def emit_vproj_tt(tt):
            ps = aux_psum.tile([128, 512], F32, tag="aux", name="ps")
            for dc in range(NDC):
                nc.tensor.matmul(ps[:, 0:256],
                                 xT_sb[dc][:, tt * 128:(tt + 1) * 128],
                                 wv_sb[dc][:, :],
                                 start=(dc == 0), stop=(dc == NDC - 1))
            vt = v_pool.tile([128, HL, 65], BF16, name=f"v{tt}", tag=f"v{tt}")
            psa = ps[:, 0:256]
            ps3 = AP(psa.tensor, psa.offset, [psa.ap[0], [64, HL], [1, 64]])
            nc.vector.tensor_copy(vt[:, :, 0:64], ps3)
            nc.vector.memset(vt[:, :, 64:65], 1.0)
            v_sb.append(vt)

        # ---------------- Phase 2: g windows via shifted-row DMA -----------------
        # g_pad holds rev[m] = score[2L-2 - m], so grepp[pp][p, h, u] =
        # rev[h-head, p + u] loads with positive strides; the multiply reads the
        # free dim reversed to recover bias[j0+p, i0+i] = rev[2047-i0+j0+p-i].
        grep_pool = ctx.enter_context(tc.tile_pool(name="grep", bufs=1))
        grepp = []
        gbase = g_pad[:]
        for pp in range(2):
            gt = grep_pool.tile([128, 2, GW], BF16, name=f"grep{pp}",
                                tag=f"grep{pp}")
            grepp.append(gt)
        for hi in range(HL):
            src = AP(gbase.tensor, gbase.offset + hi * M,
                     [[1, 128], [1, GW]])
            dma(hi, grepp[hi // 2][:, hi % 2:hi % 2 + 1, :], src)

        # ---------------- Phase 3: attention units -------------------------------
        p_pool = ctx.enter_context(tc.tile_pool(name="p", bufs=3))
        e_pool = ctx.enter_context(tc.tile_pool(name="es", bufs=3))
        o_pool = ctx.enter_context(tc.tile_pool(name="o", bufs=2))
        r_pool = ctx.enter_context(tc.tile_pool(name="r", bufs=2))
        out_pool = ctx.enter_context(tc.tile_pool(name="out", bufs=2))

        QK_SCALE = 0.125  # 1/sqrt(HD); applied inside the ACT exp

        def emit_S(pp, c, jt):
            """Row-packed head-pair S matmuls -> [128, 2, 512] PSUM tile."""
            ps = s_psum.tile([128, 2, IC], F32, tag="S", name=f"s{pp}_{c}_{jt}")
            kqt = kq_sb[pp][jt // 4]
            qqt = kq_sb[2 + pp][c]
            joff = (jt % 4) * JT
            for h in range(2):
                pb = h * 64
                nc.tensor.matmul(ps[:, h:h + 1, :],
                                 kqt[pb:pb + 64, joff:joff + JT],
                                 qqt[pb:pb + 64, :],
                                 start=True, stop=True)
            return ps

        def emit_expmult(pp, c, jt, ps):
            es = e_pool.tile([128, 2, IC], BF16, tag="es", name=f"e{pp}_{c}_{jt}")
            nc.scalar.activation(es[:, :, :], ps[:, :, :],
                                 mybir.ActivationFunctionType.Exp,
                                 scale=QK_SCALE)
            pt = p_pool.tile([128, 2, IC], BF16, tag="p", name=f"p{pp}_{c}_{jt}")
            u0 = (L - 1) - c * IC + jt * JT
            g2 = grepp[pp][:, :, :]
            g_rev = AP(g2.tensor, g2.offset + u0, [g2.ap[0], [GW, 2], [-1, IC]])
            nc.vector.tensor_mul(pt[:, :, :], es[:, :, :], g_rev)
            return pt

        psos = {}

        def emit_AV(pp, c, jt, pt):
            if jt == 0:
                psos[0] = o_psum.tile([65, IC], F32, tag="O0", name=f"o0_{pp}_{c}")
                psos[1] = o_psum.tile([65, IC], F32, tag="O1", name=f"o1_{pp}_{c}")
            for h in range(2):
                hi = 2 * pp + h
                nc.tensor.matmul(psos[h][:, :],
                                 v_sb[jt][:, hi:hi + 1, :],
                                 pt[:, h:h + 1, :],
                                 start=(jt == 0), stop=(jt == NJT - 1))
            if jt == NJT - 1:
                emit_epilogue(pp, c)

        def emit_epilogue(pp, c):
            i0 = c * IC
            for h in range(2):
                hi = 2 * pp + h
                o_sb = o_pool.tile([65, IC], F32, tag="osb", name=f"ob{pp}_{c}_{h}")
                nc.vector.tensor_copy(o_sb[:, :], psos[h][:, :])
                # 4 transposed [128, 65] chunks packed into one aux psum bank
                ps_t = aux_psum.tile([128, 512], F32, tag="aux", name="ps_t")
                for tq in range(IC // 128):
                    nc.tensor.transpose(ps_t[:, tq * 65:tq * 65 + 65],
                                        o_sb[:, tq * 128:(tq + 1) * 128],
                                        ident[0:65, 0:65])
                rc = r_pool.tile([128, 4], F32, tag="rc", name="rc")
                pst = ps_t[:, :]
                den = AP(pst.tensor, pst.offset + 64, [pst.ap[0], [65, 4]])
                nc.vector.reciprocal(rc[:, 0:4], den)
                ot = out_pool.tile([128, 4, HD], F32, tag="ot", name="ot")
                for tq in range(IC // 128):
                    nc.vector.tensor_scalar(ot[:, tq:tq + 1, :],
                                            ps_t[:, tq * 65:tq * 65 + 64],
                                            rc[:, tq:tq + 1], None,
                                            op0=mybir.AluOpType.mult)
                ob = out_d[:, :]
                dst = AP(ob.tensor, ob.offset + i0 * 256 + hi * HD,
                         [[256, 128], [128 * 256, 4], [1, HD]])
                dma(hi + c, dst, ot[:, :, :])

        # k and q projections for head-pair 0 come first (gate the first unit);
        # remaining projections interleave into the unit stream.
        for tcn in range(4):
            emit_kq_tcn(0, tcn, "act")
        for tcn in range(4):
            emit_kq_tcn(2, tcn, "act")

        units = [(pp, c, jt) for pp in range(2) for c in range(NCH)
                 for jt in range(NJT)]
        prev = None
        for idx, (pp, c, jt) in enumerate(units):
            ps = emit_S(pp, c, jt)
            # interleaved late projections
            if pp == 0 and c == 0:
                emit_vproj_tt(jt)
            elif pp == 0 and c == 1 and jt % 4 == 0:
                emit_kq_tcn(1, jt // 4, "dve")
            elif pp == 0 and c == 2 and jt % 4 == 0:
                emit_kq_tcn(3, jt // 4, "dve")
            pt = emit_expmult(pp, c, jt, ps)
            if prev is not None:
                emit_AV(*prev)
            prev = (pp, c, jt, pt)
        emit_AV(*prev)

    nc.compile()
    return nc


def shard_inputs(inputs: dict) -> list[dict]:
    """Full inputs -> 8 per-core input maps (bf16 prep for matmul operands)."""
    import ml_dtypes

    x, w_in = inputs["x"], inputs["w_in"]
    off = inputs["kernel_offsets"]
    amp = inputs["kernel_amplitudes"]
    sh = inputs["kernel_sharpness"]
    D = DM
    in_maps = []
    for c in range(8):
        b, hg = c // 4, c % 4
        heads = list(range(4 * hg, 4 * hg + 4))
        xT = np.ascontiguousarray(x[b].T).astype(ml_dtypes.bfloat16)
        rows_k = np.concatenate([w_in[h * HD:(h + 1) * HD] for h in heads])
        rows_q = np.concatenate(
            [w_in[2 * D + h * HD:2 * D + (h + 1) * HD] for h in heads])
        rows_v = np.concatenate([w_in[D + h * HD:D + (h + 1) * HD] for h in heads])
        wkqv = np.ascontiguousarray(
            np.concatenate([np.concatenate([rows_k, rows_q]).T, rows_v.T],
                           axis=1)).astype(ml_dtypes.bfloat16)
        tisa = np.zeros((64, 6), np.float32)
        tisa[:, 0] = off[heads].reshape(-1)
        tisa[:, 1] = sh[heads].reshape(-1)
        for hi in range(4):
            tisa[hi * 16:(hi + 1) * 16, 2 + hi] = amp[heads[hi]]
        in_maps.append({"xT": xT, "wkqv": wkqv, "tisa": tisa})
    return in_maps


def unshard_output(results: list[dict]) -> np.ndarray:
    out = np.zeros((2, L, DM), np.float32)
    for c in range(8):
        b, hg = c // 4, c % 4
        out[b, :, hg * 256:(hg + 1) * 256] = results[c]["out"]
    return out


_NC_CACHE = None


def kernel(**inputs) -> np.ndarray:
    global _NC_CACHE
    from concourse.bass_utils import run_bass_kernel_spmd

    if _NC_CACHE is None:
        _NC_CACHE = build_kernel()
    in_maps = shard_inputs({k: np.asarray(v) for k, v in inputs.items()})
    res = run_bass_kernel_spmd(_NC_CACHE, in_maps, core_ids=list(range(8)))
    return unshard_output(res.results)
